# revision 3
# baseline (speedup 1.0000x reference)
"""Trainium2 Bass kernel for nn_Net_39230231281866 (dense_cnn).

Network: conv3x3(1->6) -> Taylor-sigmoid -> conv3x3(6->7) -> flatten
         -> fc(4032->128) -> sigmoid -> fc(128->10) -> log_softmax,
batch 8192, data-parallel over 8 NeuronCores (1024 samples/core).

Mapping (v2):
  * conv2+fc1 folded on the host into one dense GEMM W_comb [128, 4056].
  * conv1 as 36 UNIFORM overlapping tiles of 3x7 output positions
    (window 5x9 -> K=45, M=3*7*6=126).  All tiles share ONE banded
    weight matrix and ONE per-partition bias column (m%6 channel map).
    Tiles run in pairs on PE row-groups {0, 64} (tile_position
    concurrency), so two matmuls stream simultaneously.
  * Input is host-pre-windowed into a [128, 18*1024] fp16 blob per core
    (pair-major, both batch slices contiguous) -> 6 big DMAs instead of
    72 small ones.
  * Taylor-sigmoid: custom DVE op den16(u) = u^4+2u^3+3u^2+3u+3 with
    u = (-conv-b1)/2 (scale folded into conv weights), then Reciprocal
    on ScalarE; s = 1.5/den16 with the 1.5 folded into W_comb.
  * GEMM software-pipelined 3 tiles behind conv so the strict in-order
    PE queue never stalls on the DVE->ACT elementwise chain.
"""

import os
import numpy as np
import ml_dtypes

_B = 8192
_NCORES = 8
_PC = _B // _NCORES          # 1024 samples per core
_SLICE = 512
_NSL = _PC // _SLICE         # 2 batch slices

# uniform conv1 tiling with overlap: rows 0,3,..,21,23 cols 0,7,14,19
_OYS = [0, 3, 6, 9, 12, 15, 18, 21, 23]
_OXS = [0, 7, 14, 19]
_NOY, _NOX = 3, 7            # output positions per tile
_KY, _KX = 5, 9              # input window per tile
_K = _KY * _KX               # 45
_M = _NOY * _NOX * 6         # 126
_NT = len(_OYS) * len(_OXS)  # 36 tiles
_NPAIR = _NT // 2            # 18 row-group pairs

_GEMM_LAG = 3                # software pipeline depth (tiles)
_N_DVE_RECIP = 0             # tiles whose reciprocal runs on DVE

LAST_RESULTS = None


def _tiles():
    return [(oy, ox) for oy in _OYS for ox in _OXS]


def _host_prep(x, w1, b1, w2, b2, fw1, fb1, fw2, fb2):
    x = np.asarray(x, np.float32)
    w1 = np.asarray(w1, np.float32); b1 = np.asarray(b1, np.float32)
    w2 = np.asarray(w2, np.float32); b2 = np.asarray(b2, np.float32)
    fw1 = np.asarray(fw1, np.float32); fb1 = np.asarray(fb1, np.float32)
    fw2 = np.asarray(fw2, np.float32); fb2 = np.asarray(fb2, np.float32)

    tiles = _tiles()
    f16 = np.float16

    # shared banded conv1 weights [45, 126], scaled by -1/2 (u = (-conv-b1)/2)
    w1pack = np.zeros((_K, _M), np.float32)
    for dy in range(_NOY):
        for dx in range(_NOX):
            for oc in range(6):
                m = (dy * _NOX + dx) * 6 + oc
                for ky in range(3):
                    for kx in range(3):
                        k = (dy + ky) * _KX + (dx + kx)
                        w1pack[k, m] = -0.5 * w1[oc, 0, ky, kx]
    # shared bias column: partition m -> channel m%6
    bpack = np.zeros((128, 1), np.float32)
    for m in range(_M):
        bpack[m, 0] = -0.5 * b1[m % 6]

    # fold conv2 + fc1 -> W_comb [128, 6*26*26] (x1.5: s = 1.5/den16), b_comb
    fw1r = fw1.reshape(128, 7, 24, 24)
    Wc = np.zeros((128, 6, 26, 26), np.float32)
    for dy in range(3):
        for dx in range(3):
            Wc[:, :, dy:dy + 24, dx:dx + 24] += np.einsum(
                "joyx,oi->jiyx", fw1r, w2[:, :, dy, dx], optimize=True)
    b_comb = fb1 + np.einsum("joyx,o->j", fw1r, b2)
    Wc_flat = (1.5 * Wc.reshape(128, 6 * 26 * 26)).astype(np.float32)

    # W_comb columns packed per tile [128, 36*128]; overlapped (duplicate)
    # output positions are owned by the first tile that produces them.
    owned = np.zeros((26, 26), bool)
    wcpack = np.zeros((128, 128 * _NT), np.float32)
    for t_i, (oy0, ox0) in enumerate(tiles):
        for dy in range(_NOY):
            for dx in range(_NOX):
                y, xq = oy0 + dy, ox0 + dx
                if owned[y, xq]:
                    continue
                owned[y, xq] = True
                for oc in range(6):
                    m = (dy * _NOX + dx) * 6 + oc
                    pos = (oc * 26 + y) * 26 + xq
                    wcpack[m, 128 * t_i:128 * t_i + 128] = Wc_flat[:, pos]
    assert owned.all()

    consts = dict(
        w1pack=None,  # built per layout below
        wcpack=wcpack.astype(f16),
        bpack=bpack,
        bcomb=b_comb.reshape(128, 1).astype(np.float32),
        fw2t=np.ascontiguousarray(fw2.T).astype(f16),                   # [128, 10]
        fb2r=np.tile(fb2.reshape(1, 10), (128, 4)).astype(np.float32),  # [128, 40]
    )
    # w1pack in SBUF layout [128, 126]: rows 0-44 and 64-108 both hold W
    w1sb = np.zeros((128, _M), np.float32)
    w1sb[0:_K, :] = w1pack
    w1sb[64:64 + _K, :] = w1pack
    consts["w1pack"] = w1sb.astype(f16)

    # pre-windowed input blob per core: [128, NPAIR*1024] fp16.
    # pair p: tile 2p rows at partitions 0..44, tile 2p+1 at 64..108;
    # columns 1024p..1024p+1023 = the core's full 1024-sample batch.
    x_pm = x.reshape(_B, 784).T.astype(f16)                             # [784, B]
    row_idx = []
    for (oy0, ox0) in tiles:
        rows = ((np.arange(_KY)[:, None] + oy0) * 28 +
                (np.arange(_KX)[None, :] + ox0)).reshape(-1)
        row_idx.append(rows)
    return x_pm, row_idx, consts, tiles


def _register_taylor_den16():
    import concourse.dve_ops as dve_ops
    if "TAYLOR_DEN16_ANT" in dve_ops._SUB_OPCODE_FOR_NAME:
        return next(o for o in dve_ops.OPS if o.name == "TAYLOR_DEN16_ANT")
    from concourse.dve_spec import Spec, Src0, C0, C1, C2

    # u = in0 + s0;  out = u^4 + 2u^3 + 3u^2 + 3u + 3  ==  (q(t)+48)/16
    u = Src0 + C0
    body = ((((u + C1) * u + C2) * u + C2) * u + C2)

    def _ref(in0, in1, s0, s1, imm2):
        xx = in0.astype(np.float32) + s0
        return (((xx + s1) * xx + imm2) * xx + imm2) * xx + imm2

    op = dve_ops.DveOp(
        "TAYLOR_DEN16_ANT",
        Spec(body=body, reference=_ref),
        subdim=False,
        uops_sha={"v3": "0d84493259836d20", "v4": "be052b2c26b42830"},
    )
    dve_ops.OPS.append(op)
    dve_ops.CUSTOM_DVE_SPECS[op.name] = op.spec
    row = max(dve_ops._SUB_OPCODE_FOR_NAME.values()) + 1
    assert row < 0x20
    dve_ops._SUB_OPCODE_FOR_NAME[op.name] = row
    return op


def _pin_exp_ln_table():
    """Make Exp and Ln resolve only to natural_log_exp_and_others so the
    log_softmax tail costs one table load instead of alternating sets."""
    import concourse.bacc as bacc
    import concourse.mybir as mybir
    if getattr(bacc, "_ant_expln_pinned", False):
        return
    orig = bacc.get_activation_tables
    AF = mybir.ActivationFunctionType

    def patched(arch):
        tabs = {k: set(v) for k, v in orig(arch).items()}
        for name, fns in tabs.items():
            if name != "natural_log_exp_and_others":
                fns.discard(AF.Exp)
                fns.discard(AF.Ln)
        return tabs

    bacc.get_activation_tables = patched
    bacc._ant_expln_pinned = True


def _act_raw(nc, out, in_, func, bias=0.0, scale=1.0):
    """Emit InstActivation directly (used for Reciprocal, which the
    nc.scalar.activation wrapper refuses; measured ~1.2e-5 rel err)."""
    import concourse.mybir as mybir
    eng = nc.scalar
    inputs = [eng.lower_ap(in_)]
    for arg in (bias, scale, 0.0):
        inputs.append(mybir.ImmediateValue(dtype=mybir.dt.float32,
                                           value=float(arg)))
    return eng.add_instruction(mybir.InstActivation(
        name=nc.get_next_instruction_name(), func=func, ins=inputs,
        outs=[eng.lower_ap(out)]))


def _build_program():
    import concourse.bacc as bacc
    import concourse.mybir as mybir
    from concourse.tile import TileContext
    from concourse.alu_op_type import AluOpType
    from concourse.dve_ops import RECIP_APPROX_FAST_CONSTS as RC
    import concourse.dve_ops as dve_ops

    f32 = mybir.dt.float32
    f16 = mybir.dt.float16
    AF = mybir.ActivationFunctionType
    taylor_den = _register_taylor_den16()
    recip_fast = next(o for o in dve_ops.OPS if o.name == "RECIPROCAL_APPROX_FAST")
    _pin_exp_ln_table()

    nc = bacc.Bacc()
    xwin = nc.declare_dram_parameter("xwin", [128, _NPAIR * _PC], f16,
                                     isOutput=False)
    wcpack_d = nc.declare_dram_parameter("wcpack", [128, 128 * _NT], f16,
                                         isOutput=False)
    w1pack_d = nc.declare_dram_parameter("w1pack", [128, _M], f16, isOutput=False)
    bpack_d = nc.declare_dram_parameter("bpack", [128, 1], f32, isOutput=False)
    bcomb_d = nc.declare_dram_parameter("bcomb", [128, 1], f32, isOutput=False)
    fw2t_d = nc.declare_dram_parameter("fw2t", [128, 10], f16, isOutput=False)
    fb2r_d = nc.declare_dram_parameter("fb2r", [128, 40], f32, isOutput=False)
    out_d = nc.declare_dram_parameter("out", [_PC, 10], f32, isOutput=True)

    _NCH = 6                       # xwin DMA chunks (3 pairs each)
    _PAIRS_PER_CH = _NPAIR // _NCH

    with TileContext(nc) as tc:
        with (
            tc.tile_pool(name="const", bufs=1) as cpool,
            tc.tile_pool(name="xw", bufs=1) as xpool,
            tc.tile_pool(name="work", bufs=4) as wpool,
            tc.tile_pool(name="cps", bufs=2, space="PSUM") as cps,
            tc.tile_pool(name="zps", bufs=2, space="PSUM") as zps,
            tc.tile_pool(name="fps", bufs=1, space="PSUM") as fps,
        ):
            w1pack_sb = cpool.tile_from(w1pack_d[:], name="w1pack_sb")
            wcpack_sb = cpool.tile_from(wcpack_d[:], name="wcpack_sb")
            bpack_sb = cpool.tile_from(bpack_d[:], name="bpack_sb")
            bcomb_sb = cpool.tile_from(bcomb_d[:], name="bcomb_sb")
            fw2t_sb = cpool.tile_from(fw2t_d[:], name="fw2t_sb")
            fb2r_sb = cpool.tile_from(fb2r_d[:], name="fb2r_sb")

            # input window chunks (3 pairs = 3072 cols each)
            xch = []
            for c in range(_NCH):
                t = xpool.tile([128, _PAIRS_PER_CH * _PC], f16, tag=f"xw{c}",
                               name=f"xw{c}", bufs=1)
                nc.sync.dma_start(
                    out=t, in_=xwin[:, c * _PAIRS_PER_CH * _PC:
                                    (c + 1) * _PAIRS_PER_CH * _PC])
                xch.append(t)

            # single-sync-wait rule: pre-observe PE-read const queues with
            # dummy 1-col matmuls; DVE/ACT-read consts with dummy touches.
            dps = fps.tile([128, 1], f32, tag="dps", name="dps", bufs=1)
            nc.tensor.matmul(dps[0:126, 0:1], w1pack_sb[0:45, 0:126],
                             w1pack_sb[0:45, 0:1], start=True, stop=True)
            nc.tensor.matmul(dps[0:128, 0:1], wcpack_sb[0:128, 0:128],
                             wcpack_sb[0:128, 0:1], start=True, stop=True)
            nc.tensor.matmul(dps[0:10, 0:1], fw2t_sb[0:128, 0:10],
                             fw2t_sb[0:128, 0:1], start=True, stop=True)
            dvescr = wpool.tile([128, 41], f32, tag="dvescr", name="dvescr",
                                bufs=1)
            nc.vector.tensor_copy(out=dvescr[:, 0:1], in_=bpack_sb[:])
            nc.vector.tensor_copy(out=dvescr[:, 1:41], in_=fb2r_sb[:])
            actscr = wpool.tile([128, 1], f32, tag="actscr", name="actscr",
                                bufs=1)
            nc.scalar.copy(out=actscr[:], in_=bcomb_sb[:])

            zs = [zps.tile([128, _SLICE], f32, tag="z", name=f"z{sl}")
                  for sl in range(_NSL)]

            recip_on_dve = set()
            if _N_DVE_RECIP:
                recip_on_dve = set(range(_NT - 2 - _N_DVE_RECIP, _NT - 2))

            ss = {}
            last_recip = [None]

            def emit_conv(t):
                p, j = t // 2, t % 2
                ch, pin = p // _PAIRS_PER_CH, p % _PAIRS_PER_CH
                cp = cps.tile([128, _PC], f32, tag="cp", name=f"cp{t}")
                for sl in range(_NSL):
                    nc.tensor.matmul(
                        cp[0:_M, sl * _SLICE:(sl + 1) * _SLICE],
                        w1pack_sb[64 * j:64 * j + _K, 0:_M],
                        xch[ch][64 * j:64 * j + _K,
                                pin * _PC + sl * _SLICE:
                                pin * _PC + (sl + 1) * _SLICE],
                        start=True, stop=True)
                q = wpool.tile([128, _PC], f32, tag="q", name=f"q{t}")
                nc.vector._custom_dve(
                    taylor_den, out=q, in0=cp,
                    s0=bpack_sb[0:128, 0:1], s1=2.0, imm2=3.0)
                s = wpool.tile([128, _PC], f16, tag="s", name=f"s{t}")
                if t in recip_on_dve:
                    ri = nc.vector._custom_dve(
                        recip_fast, out=s, in0=q,
                        s0=RC["s0"], s1=RC["s1"], imm2=RC["imm2"])
                else:
                    ri = _act_raw(nc, s, q, AF.Reciprocal)
                last_recip[0] = ri
                ss[t] = s

            def emit_gemm(t):
                s = ss.pop(t)
                for sl in range(_NSL):
                    nc.tensor.matmul(
                        zs[sl], wcpack_sb[0:128, 128 * t:128 * t + 128],
                        s[:, sl * _SLICE:(sl + 1) * _SLICE],
                        start=(t == 0), stop=(t == _NT - 1))

            for t in range(_NT):
                emit_conv(t)
                if t >= _GEMM_LAG:
                    emit_gemm(t - _GEMM_LAG)
            for t in range(_NT - _GEMM_LAG, _NT):
                emit_gemm(t)

            # ---- tail: sigmoid, fc2, log_softmax (no max-sub: |logits| < 12,
            # exp cannot overflow fp32).
            hs = []
            for sl in range(_NSL):
                h = wpool.tile([128, _SLICE], f16, tag="h", name=f"h{sl}")
                nc.scalar.activation(h, zs[sl], AF.Sigmoid, bias=bcomb_sb[:],
                                     scale=1.0)
                hs.append(h)
            for sl in range(_NSL):
                ng = _SLICE // 128
                fp = fps.tile([128, 10 * ng], f32, tag="fp", name=f"fp{sl}",
                              bufs=1)
                for g in range(ng):
                    nc.tensor.matmul(fp[:, g * 10:(g + 1) * 10],
                                     hs[sl][:, g * 128:(g + 1) * 128],
                                     fw2t_sb[:], start=True, stop=True)
                lg = wpool.tile([128, 10 * ng], f32, tag="lg", name=f"lg{sl}")
                nc.vector.tensor_tensor(out=lg, in0=fp, in1=fb2r_sb[:, 0:10 * ng],
                                        op=AluOpType.add)
                e = wpool.tile([128, 10 * ng], f32, tag="e", name=f"e{sl}")
                nc.scalar.activation(e, lg, AF.Exp)
                ssum = wpool.tile([128, ng], f32, tag="ss", name=f"ss{sl}")
                nc.vector.tensor_reduce(
                    ssum, e.rearrange("p (g k) -> p g k", k=10),
                    axis=mybir.AxisListType.X, op=AluOpType.add)
                lns = wpool.tile([128, ng], f32, tag="ls", name=f"ls{sl}")
                nc.scalar.activation(lns, ssum, AF.Ln)
                ot = wpool.tile([128, 10 * ng], f32, tag="ot", name=f"ot{sl}")
                for g in range(ng):
                    nc.vector.tensor_scalar(
                        out=ot[:, g * 10:(g + 1) * 10],
                        in0=lg[:, g * 10:(g + 1) * 10],
                        scalar1=lns[:, g:g + 1], scalar2=None,
                        op0=AluOpType.subtract)
                orow = sl * _SLICE
                nc.sync.dma_start(
                    out=out_d[orow:orow + _SLICE, :].rearrange(
                        "(g p) k -> p g k", p=128),
                    in_=ot.rearrange("p (g k) -> p g k", k=10))
    nc.compile()
    return nc


_PROGRAM_CACHE = {}


def kernel(x, w1, b1, w2, b2, fw1, fb1, fw2, fb2):
    global LAST_RESULTS
    x_pm, row_idx, consts, tiles = _host_prep(
        x, w1, b1, w2, b2, fw1, fb1, fw2, fb2)

    if "nc" not in _PROGRAM_CACHE:
        _PROGRAM_CACHE["nc"] = _build_program()
    nc = _PROGRAM_CACHE["nc"]

    shared = {k: consts[k] for k in
              ("wcpack", "w1pack", "bpack", "bcomb", "fw2t", "fb2r")}
    in_maps = []
    for c in range(_NCORES):
        m = dict(shared)
        xc = x_pm[:, c * _PC:(c + 1) * _PC]                 # [784, 1024]
        blob = np.zeros((128, _NPAIR * _PC), np.float16)
        for p in range(_NPAIR):
            for j in range(2):
                rows = row_idx[2 * p + j]
                blob[64 * j:64 * j + _K, p * _PC:(p + 1) * _PC] = xc[rows, :]
        m["xwin"] = blob
        in_maps.append(m)

    from concourse.bass_utils import run_bass_kernel_spmd
    trace = bool(int(os.environ.get("BASS_KERNEL_TRACE", "0")))
    res = run_bass_kernel_spmd(nc, in_maps, core_ids=list(range(_NCORES)),
                               trace=trace)
    LAST_RESULTS = res
    return np.concatenate([r["out"] for r in res.results], axis=0)


# revision 10
# speedup vs baseline: 1.2650x; 1.2650x over previous
"""Trainium2 Bass kernel for nn_Net_39230231281866 (dense_cnn).

Network: conv3x3(1->6) -> Taylor-sigmoid -> conv3x3(6->7) -> flatten
         -> fc(4032->128) -> sigmoid -> fc(128->10) -> log_softmax,
batch 8192, data-parallel over 8 NeuronCores (1024 samples/core).

Mapping (v2):
  * conv2+fc1 folded on the host into one dense GEMM W_comb [128, 4056].
  * conv1 as 36 UNIFORM overlapping tiles of 3x7 output positions
    (window 5x9 -> K=45, M=3*7*6=126).  All tiles share ONE banded
    weight matrix and ONE per-partition bias column (m%6 channel map).
    Tiles run in pairs on PE row-groups {0, 64} (tile_position
    concurrency), so two matmuls stream simultaneously.
  * Input is host-pre-windowed into a [128, 18*1024] fp16 blob per core
    (pair-major, both batch slices contiguous) -> 6 big DMAs instead of
    72 small ones.
  * Taylor-sigmoid: custom DVE op den16(u) = u^4+2u^3+3u^2+3u+3 with
    u = (-conv-b1)/2 (scale folded into conv weights), then Reciprocal
    on ScalarE; s = 1.5/den16 with the 1.5 folded into W_comb.
  * GEMM software-pipelined 3 tiles behind conv so the strict in-order
    PE queue never stalls on the DVE->ACT elementwise chain.
"""

import os
import numpy as np
import ml_dtypes

_B = 8192
_NCORES = 8
_PC = _B // _NCORES          # 1024 samples per core
_SLICE = 512
_NSL = _PC // _SLICE         # 2 batch slices

# uniform conv1 tiling with overlap: rows 0,3,..,21,23 cols 0,7,14,19
_OYS = [0, 3, 6, 9, 12, 15, 18, 21, 23]
_OXS = [0, 7, 14, 19]
_NOY, _NOX = 3, 7            # output positions per tile
_KY, _KX = 5, 9              # input window per tile
_K = _KY * _KX               # 45
_M = _NOY * _NOX * 6         # 126
_NT = len(_OYS) * len(_OXS)  # 36 tiles
_NPAIR = _NT // 2            # 18 row-group pairs

_GEMM_LAG = 2                # software pipeline depth (pairs)
_WCSCALE = 64.0              # fp8 scale for W_comb; undone in the sigmoid

LAST_RESULTS = None


def _tiles():
    return [(oy, ox) for oy in _OYS for ox in _OXS]


def _host_prep(x, w1, b1, w2, b2, fw1, fb1, fw2, fb2):
    x = np.asarray(x, np.float32)
    w1 = np.asarray(w1, np.float32); b1 = np.asarray(b1, np.float32)
    w2 = np.asarray(w2, np.float32); b2 = np.asarray(b2, np.float32)
    fw1 = np.asarray(fw1, np.float32); fb1 = np.asarray(fb1, np.float32)
    fw2 = np.asarray(fw2, np.float32); fb2 = np.asarray(fb2, np.float32)

    tiles = _tiles()
    f16 = np.float16

    # shared banded conv1 weights [45, 126], scaled by -1/2 (u = (-conv-b1)/2)
    w1pack = np.zeros((_K, _M), np.float32)
    for dy in range(_NOY):
        for dx in range(_NOX):
            for oc in range(6):
                m = (dy * _NOX + dx) * 6 + oc
                for ky in range(3):
                    for kx in range(3):
                        k = (dy + ky) * _KX + (dx + kx)
                        w1pack[k, m] = -0.5 * w1[oc, 0, ky, kx]
    # shared bias column: partition m -> channel m%6
    bpack = np.zeros((128, 1), np.float32)
    for m in range(_M):
        bpack[m, 0] = -0.5 * b1[m % 6]

    # fold conv2 + fc1 -> W_comb [128, 6*26*26] (x1.5: s = 1.5/den16), b_comb
    fw1r = fw1.reshape(128, 7, 24, 24)
    Wc = np.zeros((128, 6, 26, 26), np.float32)
    for dy in range(3):
        for dx in range(3):
            Wc[:, :, dy:dy + 24, dx:dx + 24] += np.einsum(
                "joyx,oi->jiyx", fw1r, w2[:, :, dy, dx], optimize=True)
    b_comb = fb1 + np.einsum("joyx,o->j", fw1r, b2)
    Wc_flat = (1.5 * Wc.reshape(128, 6 * 26 * 26)).astype(np.float32)

    # W_comb columns packed per tile [128, 36*128]; overlapped (duplicate)
    # output positions are owned by the first tile that produces them.
    owned = np.zeros((26, 26), bool)
    wcpack = np.zeros((128, 128 * _NT), np.float32)
    for t_i, (oy0, ox0) in enumerate(tiles):
        for dy in range(_NOY):
            for dx in range(_NOX):
                y, xq = oy0 + dy, ox0 + dx
                if owned[y, xq]:
                    continue
                owned[y, xq] = True
                for oc in range(6):
                    m = (dy * _NOX + dx) * 6 + oc
                    pos = (oc * 26 + y) * 26 + xq
                    wcpack[m, 128 * t_i:128 * t_i + 128] = Wc_flat[:, pos]
    assert owned.all()

    # DoubleRow stationary layout: per pair p -> [128, 2*128] fp8,
    # cols [j*128 + m] = wcpack block of tile 2p+j, scaled by _WCSCALE
    f8 = ml_dtypes.float8_e4m3fn
    wcpk8 = np.zeros((128, 256 * (_NT // 2)), f8)
    for p in range(_NT // 2):
        for j in range(2):
            t_i = 2 * p + j
            wcpk8[:, 256 * p + 128 * j:256 * p + 128 * j + 128] = (
                _WCSCALE * wcpack[:, 128 * t_i:128 * t_i + 128]).astype(f8)

    consts = dict(
        w1pack=None,  # built per layout below
        wcpack=wcpk8,
        bpack=bpack,
        bcomb=b_comb.reshape(128, 1).astype(np.float32),
        fw2t=np.ascontiguousarray(fw2.T).astype(f16),                   # [128, 10]
        fb2r=np.tile(fb2.reshape(1, 10), (128, 4)).astype(np.float32),  # [128, 40]
    )
    # w1pack in SBUF layout [128, 126]: rows 0-44 and 64-108 both hold W
    w1sb = np.zeros((128, _M), np.float32)
    w1sb[0:_K, :] = w1pack
    w1sb[64:64 + _K, :] = w1pack
    consts["w1pack"] = w1sb.astype(f16)

    # pre-windowed input blob per core: [128, NPAIR*1024] fp16.
    # pair p: tile 2p rows at partitions 0..44, tile 2p+1 at 64..108;
    # columns 1024p..1024p+1023 = the core's full 1024-sample batch.
    x_pm = x.reshape(_B, 784).T.astype(f16)                             # [784, B]
    row_idx = []
    for (oy0, ox0) in tiles:
        rows = ((np.arange(_KY)[:, None] + oy0) * 28 +
                (np.arange(_KX)[None, :] + ox0)).reshape(-1)
        row_idx.append(rows)
    return x_pm, row_idx, consts, tiles


def _register_taylor_den16():
    import concourse.dve_ops as dve_ops
    if "TAYLOR_DEN16_ANT" in dve_ops._SUB_OPCODE_FOR_NAME:
        return next(o for o in dve_ops.OPS if o.name == "TAYLOR_DEN16_ANT")
    from concourse.dve_spec import Spec, Src0, C0, C1, C2

    # u = in0 + s0;  out = u^4 + 2u^3 + 3u^2 + 3u + 3  ==  (q(t)+48)/16
    u = Src0 + C0
    body = ((((u + C1) * u + C2) * u + C2) * u + C2)

    def _ref(in0, in1, s0, s1, imm2):
        xx = in0.astype(np.float32) + s0
        return (((xx + s1) * xx + imm2) * xx + imm2) * xx + imm2

    op = dve_ops.DveOp(
        "TAYLOR_DEN16_ANT",
        Spec(body=body, reference=_ref),
        subdim=False,
        uops_sha={"v3": "0d84493259836d20", "v4": "be052b2c26b42830"},
    )
    dve_ops.OPS.append(op)
    dve_ops.CUSTOM_DVE_SPECS[op.name] = op.spec
    row = max(dve_ops._SUB_OPCODE_FOR_NAME.values()) + 1
    assert row < 0x20
    dve_ops._SUB_OPCODE_FOR_NAME[op.name] = row
    return op


def _pin_exp_ln_table():
    """Make Exp and Ln resolve only to natural_log_exp_and_others so the
    log_softmax tail costs one table load instead of alternating sets."""
    import concourse.bacc as bacc
    import concourse.mybir as mybir
    if getattr(bacc, "_ant_expln_pinned", False):
        return
    orig = bacc.get_activation_tables
    AF = mybir.ActivationFunctionType

    def patched(arch):
        tabs = {k: set(v) for k, v in orig(arch).items()}
        for name, fns in tabs.items():
            if name != "natural_log_exp_and_others":
                fns.discard(AF.Exp)
                fns.discard(AF.Ln)
        return tabs

    bacc.get_activation_tables = patched
    bacc._ant_expln_pinned = True


def _act_raw(nc, out, in_, func, bias=0.0, scale=1.0):
    """Emit InstActivation directly (used for Reciprocal, which the
    nc.scalar.activation wrapper refuses; measured ~1.2e-5 rel err)."""
    import concourse.mybir as mybir
    eng = nc.scalar
    inputs = [eng.lower_ap(in_)]
    for arg in (bias, scale, 0.0):
        inputs.append(mybir.ImmediateValue(dtype=mybir.dt.float32,
                                           value=float(arg)))
    return eng.add_instruction(mybir.InstActivation(
        name=nc.get_next_instruction_name(), func=func, ins=inputs,
        outs=[eng.lower_ap(out)]))


def _build_program():
    import concourse.bacc as bacc
    import concourse.mybir as mybir
    from concourse.tile import TileContext
    from concourse.alu_op_type import AluOpType
    from concourse.dve_ops import RECIP_APPROX_FAST_CONSTS as RC
    import concourse.dve_ops as dve_ops

    f32 = mybir.dt.float32
    f16 = mybir.dt.float16
    AF = mybir.ActivationFunctionType
    taylor_den = _register_taylor_den16()
    recip_fast = next(o for o in dve_ops.OPS if o.name == "RECIPROCAL_APPROX_FAST")
    _pin_exp_ln_table()

    f8 = mybir.dt.float8e4
    nc = bacc.Bacc()
    xwin = nc.declare_dram_parameter("xwin", [128, _NPAIR * _PC], f8,
                                     isOutput=False)
    wcpack_d = nc.declare_dram_parameter("wcpack", [128, 256 * _NPAIR], f8,
                                         isOutput=False)
    w1pack_d = nc.declare_dram_parameter("w1pack", [128, _M], f16, isOutput=False)
    bpack_d = nc.declare_dram_parameter("bpack", [128, 1], f32, isOutput=False)
    bcomb_d = nc.declare_dram_parameter("bcomb", [128, 1], f32, isOutput=False)
    fw2t_d = nc.declare_dram_parameter("fw2t", [128, 10], f16, isOutput=False)
    fb2r_d = nc.declare_dram_parameter("fb2r", [128, 40], f32, isOutput=False)
    out_d = nc.declare_dram_parameter("out", [_PC, 10], f32, isOutput=True)

    _NCH = 6                       # xwin DMA chunks (3 pairs each)
    _PAIRS_PER_CH = _NPAIR // _NCH

    with TileContext(nc) as tc:
        with (
            tc.tile_pool(name="const", bufs=1) as cpool,
            tc.tile_pool(name="xw", bufs=1) as xpool,
            tc.tile_pool(name="work", bufs=4) as wpool,
            tc.tile_pool(name="cps", bufs=2, space="PSUM") as cps,
            tc.tile_pool(name="zps", bufs=2, space="PSUM") as zps,
            tc.tile_pool(name="fps", bufs=1, space="PSUM") as fps,
        ):
            # input window chunks first (3 pairs = 3072 cols each) so the
            # first conv can start as early as possible
            xch = []
            xts = []
            for c in range(_NCH):
                t = xpool.tile([128, _PAIRS_PER_CH * _PC], f8, tag=f"xw{c}",
                               name=f"xw{c}", bufs=1)
                xts.append(t)
                xch.append(t)
            nc.sync.dma_start(out=xts[0],
                              in_=xwin[:, 0:_PAIRS_PER_CH * _PC])
            w1pack_sb = cpool.tile_from(w1pack_d[:], name="w1pack_sb")
            bpack_sb = cpool.tile_from(bpack_d[:], name="bpack_sb")
            for c in range(1, _NCH):
                nc.sync.dma_start(
                    out=xts[c], in_=xwin[:, c * _PAIRS_PER_CH * _PC:
                                         (c + 1) * _PAIRS_PER_CH * _PC])
            wcpack_sb = cpool.tile_from(wcpack_d[:], name="wcpack_sb")
            bcomb_sb = cpool.tile_from(bcomb_d[:], name="bcomb_sb")
            fw2t_sb = cpool.tile_from(fw2t_d[:], name="fw2t_sb")
            fb2r_sb = cpool.tile_from(fb2r_d[:], name="fb2r_sb")

            # single-sync-wait rule: pre-observe PE-read const queues with
            # dummy 1-col matmuls; DVE/ACT-read consts with dummy touches.
            dps = fps.tile([128, 1], f32, tag="dps", name="dps", bufs=1)
            nc.tensor.matmul(dps[0:126, 0:1], w1pack_sb[0:45, 0:126],
                             w1pack_sb[0:45, 0:1], start=True, stop=True)
            nc.tensor.matmul(dps[0:128, 0:1], wcpack_sb[0:128, 0:128],
                             wcpack_sb[0:128, 0:1], start=True, stop=True)
            nc.tensor.matmul(dps[0:10, 0:1], fw2t_sb[0:128, 0:10],
                             fw2t_sb[0:128, 0:1], start=True, stop=True)
            dvescr = wpool.tile([128, 41], f32, tag="dvescr", name="dvescr",
                                bufs=1)
            nc.vector.tensor_copy(out=dvescr[:, 0:1], in_=bpack_sb[:])
            nc.vector.tensor_copy(out=dvescr[:, 1:41], in_=fb2r_sb[:])
            actscr = wpool.tile([128, 1], f32, tag="actscr", name="actscr",
                                bufs=1)
            nc.scalar.copy(out=actscr[:], in_=bcomb_sb[:])

            zs = [zps.tile([128, _SLICE], f32, tag="z", name=f"z{sl}")
                  for sl in range(_NSL)]

            ss = {}

            def emit_conv(t, qp):
                p, j = t // 2, t % 2
                ch, pin = p // _PAIRS_PER_CH, p % _PAIRS_PER_CH
                cp = cps.tile([128, _PC], f32, tag="cp", name=f"cp{t}")
                for sl in range(_NSL):
                    nc.tensor.matmul(
                        cp[0:_M, sl * _SLICE:(sl + 1) * _SLICE],
                        w1pack_sb[64 * j:64 * j + _K, 0:_M],
                        xch[ch][64 * j:64 * j + _K,
                                pin * _PC + sl * _SLICE:
                                pin * _PC + (sl + 1) * _SLICE],
                        start=True, stop=True)
                nc.vector._custom_dve(
                    taylor_den, out=qp[:, j * _PC:(j + 1) * _PC], in0=cp,
                    s0=bpack_sb[0:128, 0:1], s1=2.0, imm2=3.0)

            def emit_pair(p):
                qp = wpool.tile([128, 2 * _PC], f32, tag="q", name=f"q{p}")
                emit_conv(2 * p, qp)
                emit_conv(2 * p + 1, qp)
                s = wpool.tile([128, 2 * _PC], f8, tag="s", name=f"s{p}")
                _act_raw(nc, s, qp, AF.Reciprocal)
                ss[p] = s

            def emit_gemm(p):
                s = ss.pop(p)
                # s layout [128, (j 2) (sl 2) (n 512)] -> DoubleRow rhs
                sv = s.rearrange("p (j a n) -> p j a n", j=2, n=_SLICE)
                wv = wcpack_sb[:, 256 * p:256 * (p + 1)].rearrange(
                    "p (j m) -> p j m", j=2)
                for sl in range(_NSL):
                    nc.tensor.matmul(
                        zs[sl], wv, sv[:, :, sl:sl + 1, :],
                        start=(p == 0), stop=(p == _NPAIR - 1),
                        perf_mode=mybir.MatmulPerfMode.DoubleRow)

            for p in range(_NPAIR):
                emit_pair(p)
                if p >= _GEMM_LAG:
                    emit_gemm(p - _GEMM_LAG)
            for p in range(_NPAIR - _GEMM_LAG, _NPAIR):
                emit_gemm(p)

            # ---- tail: sigmoid, fc2, log_softmax (no max-sub: |logits| < 12,
            # exp cannot overflow fp32).
            hs = []
            for sl in range(_NSL):
                h = wpool.tile([128, _SLICE], f16, tag="h", name=f"h{sl}")
                nc.scalar.activation(h, zs[sl], AF.Sigmoid, bias=bcomb_sb[:],
                                     scale=1.0 / _WCSCALE)
                hs.append(h)
            for sl in range(_NSL):
                ng = _SLICE // 128
                fp = fps.tile([128, 10 * ng], f32, tag="fp", name=f"fp{sl}",
                              bufs=1)
                for g in range(ng):
                    nc.tensor.matmul(fp[:, g * 10:(g + 1) * 10],
                                     hs[sl][:, g * 128:(g + 1) * 128],
                                     fw2t_sb[:], start=True, stop=True)
                lg = wpool.tile([128, 10 * ng], f32, tag="lg", name=f"lg{sl}")
                nc.vector.tensor_tensor(out=lg, in0=fp, in1=fb2r_sb[:, 0:10 * ng],
                                        op=AluOpType.add)
                e = wpool.tile([128, 10 * ng], f32, tag="e", name=f"e{sl}")
                nc.scalar.activation(e, lg, AF.Exp)
                ssum = wpool.tile([128, ng], f32, tag="ss", name=f"ss{sl}")
                nc.vector.tensor_reduce(
                    ssum, e.rearrange("p (g k) -> p g k", k=10),
                    axis=mybir.AxisListType.X, op=AluOpType.add)
                lns = wpool.tile([128, ng], f32, tag="ls", name=f"ls{sl}")
                nc.scalar.activation(lns, ssum, AF.Ln)
                ot = wpool.tile([128, 10 * ng], f32, tag="ot", name=f"ot{sl}")
                for g in range(ng):
                    nc.vector.tensor_scalar(
                        out=ot[:, g * 10:(g + 1) * 10],
                        in0=lg[:, g * 10:(g + 1) * 10],
                        scalar1=lns[:, g:g + 1], scalar2=None,
                        op0=AluOpType.subtract)
                orow = sl * _SLICE
                nc.sync.dma_start(
                    out=out_d[orow:orow + _SLICE, :].rearrange(
                        "(g p) k -> p g k", p=128),
                    in_=ot.rearrange("p (g k) -> p g k", k=10))
    nc.compile()
    return nc


_PROGRAM_CACHE = {}


def kernel(x, w1, b1, w2, b2, fw1, fb1, fw2, fb2):
    global LAST_RESULTS
    x_pm, row_idx, consts, tiles = _host_prep(
        x, w1, b1, w2, b2, fw1, fb1, fw2, fb2)

    if "nc" not in _PROGRAM_CACHE:
        _PROGRAM_CACHE["nc"] = _build_program()
    nc = _PROGRAM_CACHE["nc"]

    shared = {k: consts[k] for k in
              ("wcpack", "w1pack", "bpack", "bcomb", "fw2t", "fb2r")}
    in_maps = []
    for c in range(_NCORES):
        m = dict(shared)
        xc = x_pm[:, c * _PC:(c + 1) * _PC]                 # [784, 1024]
        blob = np.zeros((128, _NPAIR * _PC), ml_dtypes.float8_e4m3fn)
        for p in range(_NPAIR):
            for j in range(2):
                rows = row_idx[2 * p + j]
                blob[64 * j:64 * j + _K, p * _PC:(p + 1) * _PC] = xc[rows, :]
        m["xwin"] = blob
        in_maps.append(m)

    from concourse.bass_utils import run_bass_kernel_spmd
    trace = bool(int(os.environ.get("BASS_KERNEL_TRACE", "0")))
    res = run_bass_kernel_spmd(nc, in_maps, core_ids=list(range(_NCORES)),
                               trace=trace)
    LAST_RESULTS = res
    return np.concatenate([r["out"] for r in res.results], axis=0)


# revision 21
# speedup vs baseline: 1.2734x; 1.0067x over previous
"""Trainium2 Bass kernel for nn_Net_39230231281866 (dense_cnn).

Network: conv3x3(1->6) -> Taylor-sigmoid -> conv3x3(6->7) -> flatten
         -> fc(4032->128) -> sigmoid -> fc(128->10) -> log_softmax,
batch 8192, data-parallel over 8 NeuronCores (1024 samples/core).

Mapping (v2):
  * conv2+fc1 folded on the host into one dense GEMM W_comb [128, 4056].
  * conv1 as 36 UNIFORM overlapping tiles of 3x7 output positions
    (window 5x9 -> K=45, M=3*7*6=126).  All tiles share ONE banded
    weight matrix and ONE per-partition bias column (m%6 channel map).
    Tiles run in pairs on PE row-groups {0, 64} (tile_position
    concurrency), so two matmuls stream simultaneously.
  * Input is host-pre-windowed into a [128, 18*1024] fp16 blob per core
    (pair-major, both batch slices contiguous) -> 6 big DMAs instead of
    72 small ones.
  * Taylor-sigmoid: custom DVE op den16(u) = u^4+2u^3+3u^2+3u+3 with
    u = (-conv-b1)/2 (scale folded into conv weights), then Reciprocal
    on ScalarE; s = 1.5/den16 with the 1.5 folded into W_comb.
  * GEMM software-pipelined 3 tiles behind conv so the strict in-order
    PE queue never stalls on the DVE->ACT elementwise chain.
"""

import os
import numpy as np
import ml_dtypes

_B = 8192
_NCORES = 8
_PC = _B // _NCORES          # 1024 samples per core
_SLICE = 512
_NSL = _PC // _SLICE         # 2 batch slices

# uniform conv1 tiling with overlap: rows 0,3,..,21,23 cols 0,7,14,19
_OYS = [0, 3, 6, 9, 12, 15, 18, 21, 23]
_OXS = [0, 7, 14, 19]
_NOY, _NOX = 3, 7            # output positions per tile
_KY, _KX = 5, 9              # input window per tile
_K = _KY * _KX               # 45
_M = _NOY * _NOX * 6         # 126
_NT = len(_OYS) * len(_OXS)  # 36 tiles
_NPAIR = _NT // 2            # 18 row-group pairs

_GEMM_LAG = 2                # software pipeline depth (pairs)
_WCSCALE = 64.0              # fp8 scale for W_comb; undone in the sigmoid

LAST_RESULTS = None


def _tiles():
    return [(oy, ox) for oy in _OYS for ox in _OXS]


def _host_prep(x, w1, b1, w2, b2, fw1, fb1, fw2, fb2):
    x = np.asarray(x, np.float32)
    w1 = np.asarray(w1, np.float32); b1 = np.asarray(b1, np.float32)
    w2 = np.asarray(w2, np.float32); b2 = np.asarray(b2, np.float32)
    fw1 = np.asarray(fw1, np.float32); fb1 = np.asarray(fb1, np.float32)
    fw2 = np.asarray(fw2, np.float32); fb2 = np.asarray(fb2, np.float32)

    tiles = _tiles()
    f16 = np.float16

    # shared banded conv1 weights [45, 126], scaled by -1/2 (u = (-conv-b1)/2)
    w1pack = np.zeros((_K + 1, _M), np.float32)   # row 45 stays zero (pad)
    for dy in range(_NOY):
        for dx in range(_NOX):
            for oc in range(6):
                m = (dy * _NOX + dx) * 6 + oc
                for ky in range(3):
                    for kx in range(3):
                        k = (dy + ky) * _KX + (dx + kx)
                        w1pack[k, m] = -0.5 * w1[oc, 0, ky, kx]
    # shared bias column: partition m -> channel m%6
    bpack = np.zeros((128, 1), np.float32)
    for m in range(_M):
        bpack[m, 0] = -0.5 * b1[m % 6]

    # fold conv2 + fc1 -> W_comb [128, 6*26*26] (x1.5: s = 1.5/den16), b_comb
    fw1r = fw1.reshape(128, 7, 24, 24)
    Wc = np.zeros((128, 6, 26, 26), np.float32)
    for dy in range(3):
        for dx in range(3):
            Wc[:, :, dy:dy + 24, dx:dx + 24] += np.einsum(
                "joyx,oi->jiyx", fw1r, w2[:, :, dy, dx], optimize=True)
    b_comb = fb1 + np.einsum("joyx,o->j", fw1r, b2)
    Wc_flat = (1.5 * Wc.reshape(128, 6 * 26 * 26)).astype(np.float32)

    # W_comb columns packed per tile [128, 36*128]; overlapped (duplicate)
    # output positions are owned by the first tile that produces them.
    owned = np.zeros((26, 26), bool)
    wcpack = np.zeros((128, 128 * _NT), np.float32)
    for t_i, (oy0, ox0) in enumerate(tiles):
        for dy in range(_NOY):
            for dx in range(_NOX):
                y, xq = oy0 + dy, ox0 + dx
                if owned[y, xq]:
                    continue
                owned[y, xq] = True
                for oc in range(6):
                    m = (dy * _NOX + dx) * 6 + oc
                    pos = (oc * 26 + y) * 26 + xq
                    wcpack[m, 128 * t_i:128 * t_i + 128] = Wc_flat[:, pos]
    assert owned.all()

    # DoubleRow stationary layout: per pair p -> [128, 2*128] fp8,
    # cols [j*128 + m] = wcpack block of tile 2p+j, scaled by _WCSCALE
    f8 = ml_dtypes.float8_e4m3fn
    wcpk8 = np.zeros((128, 256 * (_NT // 2)), f8)
    for p in range(_NT // 2):
        for j in range(2):
            t_i = 2 * p + j
            wcpk8[:, 256 * p + 128 * j:256 * p + 128 * j + 128] = (
                _WCSCALE * wcpack[:, 128 * t_i:128 * t_i + 128]).astype(f8)

    consts = dict(
        w1pack=None,  # built per layout below
        wcpack=wcpk8,
        bpack=bpack,
        bcomb=b_comb.reshape(128, 1).astype(np.float32),
        fw2t=np.ascontiguousarray(fw2.T).astype(f16),                   # [128, 10]
        fb2r=np.tile(fb2.reshape(1, 10), (128, 4)).astype(np.float32),  # [128, 40]
    )
    # w1pack in DoubleRow SBUF layout [128, 252] fp8: at each row group
    # 32g, partition k (k<23), col h*126+m = w1pack[h*23+k, m]
    _KH = 23
    w1dr = np.zeros((_KH, 256), np.float32)     # [k, (h, m-padded-128)]
    for h in range(2):
        for k in range(_KH):
            r = h * _KH + k
            if r <= _K:
                w1dr[k, h * 128:h * 128 + _M] = w1pack[r, :]
    w1sb = np.zeros((128, 256), np.float32)
    for g in range(4):
        w1sb[32 * g:32 * g + _KH, :] = w1dr
    consts["w1pack"] = w1sb.astype(ml_dtypes.float8_e4m3fn)

    # window pixel indices per tile (46 rows, last is a zero pad slot)
    x_pm = x.reshape(_B, 784).T.astype(np.float32)                      # [784, B]
    row_idx = []
    for (oy0, ox0) in tiles:
        rows = ((np.arange(_KY)[:, None] + oy0) * 28 +
                (np.arange(_KX)[None, :] + ox0)).reshape(-1)
        row_idx.append(rows)
    return x_pm, row_idx, consts, tiles


def _register_taylor_den16():
    import concourse.dve_ops as dve_ops
    if "TAYLOR_DEN16_ANT" in dve_ops._SUB_OPCODE_FOR_NAME:
        return next(o for o in dve_ops.OPS if o.name == "TAYLOR_DEN16_ANT")
    from concourse.dve_spec import Spec, Src0, C0, C1, C2

    # u = in0 + s0;  out = u^4 + 2u^3 + 3u^2 + 3u + 3  ==  (q(t)+48)/16
    u = Src0 + C0
    body = ((((u + C1) * u + C2) * u + C2) * u + C2)

    def _ref(in0, in1, s0, s1, imm2):
        xx = in0.astype(np.float32) + s0
        return (((xx + s1) * xx + imm2) * xx + imm2) * xx + imm2

    op = dve_ops.DveOp(
        "TAYLOR_DEN16_ANT",
        Spec(body=body, reference=_ref),
        subdim=False,
        uops_sha={"v3": "0d84493259836d20", "v4": "be052b2c26b42830"},
    )
    dve_ops.OPS.append(op)
    dve_ops.CUSTOM_DVE_SPECS[op.name] = op.spec
    row = max(dve_ops._SUB_OPCODE_FOR_NAME.values()) + 1
    assert row < 0x20
    dve_ops._SUB_OPCODE_FOR_NAME[op.name] = row
    return op


def _pin_exp_ln_table():
    """Make Exp and Ln resolve only to natural_log_exp_and_others so the
    log_softmax tail costs one table load instead of alternating sets."""
    import concourse.bacc as bacc
    import concourse.mybir as mybir
    if getattr(bacc, "_ant_expln_pinned", False):
        return
    orig = bacc.get_activation_tables
    AF = mybir.ActivationFunctionType

    def patched(arch):
        tabs = {k: set(v) for k, v in orig(arch).items()}
        for name, fns in tabs.items():
            if name != "natural_log_exp_and_others":
                fns.discard(AF.Exp)
                fns.discard(AF.Ln)
        return tabs

    bacc.get_activation_tables = patched
    bacc._ant_expln_pinned = True


def _act_raw(nc, out, in_, func, bias=0.0, scale=1.0):
    """Emit InstActivation directly (used for Reciprocal, which the
    nc.scalar.activation wrapper refuses; measured ~1.2e-5 rel err)."""
    import concourse.mybir as mybir
    eng = nc.scalar
    inputs = [eng.lower_ap(in_)]
    for arg in (bias, scale, 0.0):
        inputs.append(mybir.ImmediateValue(dtype=mybir.dt.float32,
                                           value=float(arg)))
    return eng.add_instruction(mybir.InstActivation(
        name=nc.get_next_instruction_name(), func=func, ins=inputs,
        outs=[eng.lower_ap(out)]))


def _build_program():
    import concourse.bacc as bacc
    import concourse.mybir as mybir
    from concourse.tile import TileContext
    from concourse.alu_op_type import AluOpType
    from concourse.dve_ops import RECIP_APPROX_FAST_CONSTS as RC
    import concourse.dve_ops as dve_ops

    f32 = mybir.dt.float32
    f16 = mybir.dt.float16
    AF = mybir.ActivationFunctionType
    taylor_den = _register_taylor_den16()
    recip_fast = next(o for o in dve_ops.OPS if o.name == "RECIPROCAL_APPROX_FAST")
    _pin_exp_ln_table()

    f8 = mybir.dt.float8e4
    nc = bacc.Bacc()
    xwin = nc.declare_dram_parameter("xwin", [128, _NPAIR * _PC], f8,
                                     isOutput=False)
    wcpack_d = nc.declare_dram_parameter("wcpack", [128, 256 * _NPAIR], f8,
                                         isOutput=False)
    w1pack_d = nc.declare_dram_parameter("w1pack", [128, 256], f8,
                                         isOutput=False)
    bpack_d = nc.declare_dram_parameter("bpack", [128, 1], f32, isOutput=False)
    bcomb_d = nc.declare_dram_parameter("bcomb", [128, 1], f32, isOutput=False)
    fw2t_d = nc.declare_dram_parameter("fw2t", [128, 10], f16, isOutput=False)
    fb2r_d = nc.declare_dram_parameter("fb2r", [128, 40], f32, isOutput=False)
    out_d = nc.declare_dram_parameter("out", [_PC, 10], f32, isOutput=True)

    # xwin DMA chunks in quads (4 tiles = 2048 cols each); first chunk
    # small so compute starts early
    _CH_QUADS = [1, 2, 3, 3]
    _CH_OFF = [0, 1, 3, 6, 9]      # cumulative quads

    with TileContext(nc) as tc:
        with (
            tc.tile_pool(name="const", bufs=1) as cpool,
            tc.tile_pool(name="xw", bufs=1) as xpool,
            tc.tile_pool(name="work", bufs=4) as wpool,
            tc.tile_pool(name="cps", bufs=2, space="PSUM") as cps,
            tc.tile_pool(name="zps", bufs=2, space="PSUM") as zps,
            tc.tile_pool(name="fps", bufs=1, space="PSUM") as fps,
        ):
            # input window chunks first so the first conv can start early
            xts = []
            for c, nq in enumerate(_CH_QUADS):
                t = xpool.tile([128, nq * 2048], f8, tag=f"xw{c}",
                               name=f"xw{c}", bufs=1)
                xts.append(t)
            nc.sync.dma_start(out=xts[0], in_=xwin[:, 0:2048])
            w1pack_sb = cpool.tile_from(w1pack_d[:], name="w1pack_sb")
            bpack_sb = cpool.tile_from(bpack_d[:], name="bpack_sb")
            for c in range(1, len(_CH_QUADS)):
                nc.sync.dma_start(
                    out=xts[c], in_=xwin[:, _CH_OFF[c] * 2048:
                                         _CH_OFF[c + 1] * 2048])
            wcpack_sb = cpool.tile_from(wcpack_d[:], name="wcpack_sb")
            bcomb_sb = cpool.tile_from(bcomb_d[:], name="bcomb_sb")
            fw2t_sb = cpool.tile_from(fw2t_d[:], name="fw2t_sb")
            fb2r_sb = cpool.tile_from(fb2r_d[:], name="fb2r_sb")

            def xw_ap(t, sl):
                """DoubleRow rhs [23, 2, 512] for tile t, slice sl."""
                quad, g = t // 4, t % 4
                for c in range(len(_CH_QUADS)):
                    if _CH_OFF[c] <= quad < _CH_OFF[c + 1]:
                        break
                base = (quad - _CH_OFF[c]) * 2048 + sl * 1024
                return xts[c][32 * g:32 * g + 23,
                              base:base + 1024].rearrange(
                                  "p (h n) -> p h n", n=_SLICE)

            # single-sync-wait rule: pre-observe PE-read const queues with
            # dummy 1-col matmuls; DVE/ACT-read consts with dummy touches.
            dps = fps.tile([128, 1], f32, tag="dps", name="dps", bufs=1)
            nc.tensor.matmul(dps[0:126, 0:1], w1pack_sb[0:23, 0:126],
                             w1pack_sb[0:23, 0:1], start=True, stop=True)
            nc.tensor.matmul(dps[0:128, 0:1], wcpack_sb[0:128, 0:128],
                             wcpack_sb[0:128, 0:1], start=True, stop=True)
            nc.tensor.matmul(dps[0:10, 0:1], fw2t_sb[0:128, 0:10],
                             fw2t_sb[0:128, 0:1], start=True, stop=True)
            dvescr = wpool.tile([128, 41], f32, tag="dvescr", name="dvescr",
                                bufs=1)
            nc.vector.tensor_copy(out=dvescr[:, 0:1], in_=bpack_sb[:])
            nc.vector.tensor_copy(out=dvescr[:, 1:41], in_=fb2r_sb[:])
            actscr = wpool.tile([128, 1], f32, tag="actscr", name="actscr",
                                bufs=1)
            nc.scalar.copy(out=actscr[:], in_=bcomb_sb[:])

            zs = [zps.tile([128, _SLICE], f32, tag="z", name=f"z{sl}")
                  for sl in range(_NSL)]

            ss = {}

            def emit_conv(t, qp, j):
                g = t % 4
                wv = w1pack_sb[32 * g:32 * g + 23, :].rearrange(
                    "p (h m) -> p h m", h=2)   # [23, 2, 128]
                cp = cps.tile([128, _PC], f32, tag="cp", name=f"cp{t}")
                for sl in range(_NSL):
                    nc.tensor.matmul(
                        cp[0:128, sl * _SLICE:(sl + 1) * _SLICE],
                        wv, xw_ap(t, sl), start=True, stop=True,
                        perf_mode=mybir.MatmulPerfMode.DoubleRow,
                        tile_position=(32 * g, 0))
                nc.vector._custom_dve(
                    taylor_den, out=qp[:, j * _PC:(j + 1) * _PC], in0=cp,
                    s0=bpack_sb[0:128, 0:1], s1=2.0, imm2=3.0)

            def emit_pair(p):
                qp = wpool.tile([128, 2 * _PC], f32, tag="q", name=f"q{p}")
                emit_conv(2 * p, qp, 0)
                emit_conv(2 * p + 1, qp, 1)
                s = wpool.tile([128, 2 * _PC], f8, tag="s", name=f"s{p}")
                _act_raw(nc, s, qp, AF.Reciprocal)
                ss[p] = s

            def emit_gemm(p):
                s = ss.pop(p)
                # s layout [128, (j 2) (sl 2) (n 512)] -> DoubleRow rhs
                sv = s.rearrange("p (j a n) -> p j a n", j=2, n=_SLICE)
                wv = wcpack_sb[:, 256 * p:256 * (p + 1)].rearrange(
                    "p (j m) -> p j m", j=2)
                for sl in range(_NSL):
                    nc.tensor.matmul(
                        zs[sl], wv, sv[:, :, sl:sl + 1, :],
                        start=(p == 0), stop=(p == _NPAIR - 1),
                        perf_mode=mybir.MatmulPerfMode.DoubleRow)

            for p in range(_NPAIR):
                emit_pair(p)
                if p >= _GEMM_LAG:
                    emit_gemm(p - _GEMM_LAG)
            for p in range(_NPAIR - _GEMM_LAG, _NPAIR):
                emit_gemm(p)

            # ---- tail: sigmoid, fc2, log_softmax (no max-sub: |logits| < 12,
            # exp cannot overflow fp32).
            hs = []
            for sl in range(_NSL):
                h = wpool.tile([128, _SLICE], f16, tag="h", name=f"h{sl}")
                nc.scalar.activation(h, zs[sl], AF.Sigmoid, bias=bcomb_sb[:],
                                     scale=1.0 / _WCSCALE)
                hs.append(h)
            for sl in range(_NSL):
                ng = _SLICE // 128
                fp = fps.tile([128, 10 * ng], f32, tag="fp", name=f"fp{sl}",
                              bufs=1)
                for g in range(ng):
                    nc.tensor.matmul(fp[:, g * 10:(g + 1) * 10],
                                     hs[sl][:, g * 128:(g + 1) * 128],
                                     fw2t_sb[:], start=True, stop=True)
                lg = wpool.tile([128, 10 * ng], f32, tag="lg", name=f"lg{sl}")
                nc.vector.tensor_tensor(out=lg, in0=fp, in1=fb2r_sb[:, 0:10 * ng],
                                        op=AluOpType.add)
                e = wpool.tile([128, 10 * ng], f32, tag="e", name=f"e{sl}")
                nc.scalar.activation(e, lg, AF.Exp)
                ssum = wpool.tile([128, ng], f32, tag="ss", name=f"ss{sl}")
                nc.vector.tensor_reduce(
                    ssum, e.rearrange("p (g k) -> p g k", k=10),
                    axis=mybir.AxisListType.X, op=AluOpType.add)
                lns = wpool.tile([128, ng], f32, tag="ls", name=f"ls{sl}")
                nc.scalar.activation(lns, ssum, AF.Ln)
                ot = wpool.tile([128, 10 * ng], f32, tag="ot", name=f"ot{sl}")
                for g in range(ng):
                    nc.vector.tensor_scalar(
                        out=ot[:, g * 10:(g + 1) * 10],
                        in0=lg[:, g * 10:(g + 1) * 10],
                        scalar1=lns[:, g:g + 1], scalar2=None,
                        op0=AluOpType.subtract)
                orow = sl * _SLICE
                nc.sync.dma_start(
                    out=out_d[orow:orow + _SLICE, :].rearrange(
                        "(g p) k -> p g k", p=128),
                    in_=ot.rearrange("p (g k) -> p g k", k=10))
    nc.compile()
    return nc


_PROGRAM_CACHE = {}


def kernel(x, w1, b1, w2, b2, fw1, fb1, fw2, fb2):
    global LAST_RESULTS
    x_pm, row_idx, consts, tiles = _host_prep(
        x, w1, b1, w2, b2, fw1, fb1, fw2, fb2)

    if "nc" not in _PROGRAM_CACHE:
        _PROGRAM_CACHE["nc"] = _build_program()
    nc = _PROGRAM_CACHE["nc"]

    shared = {k: consts[k] for k in
              ("wcpack", "w1pack", "bpack", "bcomb", "fw2t", "fb2r")}
    in_maps = []
    for c in range(_NCORES):
        m = dict(shared)
        xc = x_pm[:, c * _PC:(c + 1) * _PC]                 # [784, 1024]
        # layout per tile t (quad=t//4, g=t%4): partitions 32g..32g+22,
        # cols 2048*quad + 1024*sl + 512*h + b  =  x[winrow h*23+k, sample]
        blob = np.zeros((128, _NPAIR * _PC), ml_dtypes.float8_e4m3fn)
        for t in range(_NT):
            quad, g = t // 4, t % 4
            rows = row_idx[t]                               # 45 pixel indices
            w = xc[rows, :]                                 # [45, 1024]
            wp = np.zeros((46, _PC), np.float32)
            wp[:45] = w
            for sl in range(_NSL):
                for h in range(2):
                    blob[32 * g:32 * g + 23,
                         2048 * quad + 1024 * sl + 512 * h:
                         2048 * quad + 1024 * sl + 512 * (h + 1)] = \
                        wp[h * 23:(h + 1) * 23,
                           sl * _SLICE:(sl + 1) * _SLICE]
        m["xwin"] = blob
        in_maps.append(m)

    from concourse.bass_utils import run_bass_kernel_spmd
    trace = bool(int(os.environ.get("BASS_KERNEL_TRACE", "0")))
    res = run_bass_kernel_spmd(nc, in_maps, core_ids=list(range(_NCORES)),
                               trace=trace)
    LAST_RESULTS = res
    return np.concatenate([r["out"] for r in res.results], axis=0)
